# revision 9
# baseline (speedup 1.0000x reference)
"""Trainium2 Bass kernel for CurvSelfAttention (B=2, S=2048, E=1024, H=16).

Sharding: 8 cores = 2 batches x 4 head-quads. Core c handles batch c//4 and
heads [4*(c%4), 4*(c%4)+4). Attention is head-independent, so there are no
collectives; each core gets its batch's hidden states and its heads' weight
row-slices, and returns a [S, 256] slice of the output.

v2: all matmul operands are bf16 (the 4-byte fp32r moving path measured
~2.2 cyc/column on silicon vs ~1 for bf16), and every transpose moved off
the PE onto the DMA XBAR engine (dma_start_transpose, 2-byte dtypes only).
ScalarE runs only sigmoid + the 128 exp tiles (the hard ~133us floor at
1.2 GHz); biases/scales/copies run on DVE/GpSimd; the group-scale
expansion s_val -> s_rep is a stride-0 broadcast DMA instead of a PE
matmul + ScalarE copy.

Per-core program:
  A. hid [2048,1024] f32 -> bf16 (DVE/GpSimd) -> hidT_all [128,8j,2048]
     via XBAR transpose; same for the four weight matrices.
  B. K/Q/s projections: stationary wT tiles, moving hidT (N=512);
     V projection: stationary hidT t-tiles, moving wvT (N=256, [t,d]
     layout for the ctx contraction). Q gets bias+group-scale fused in
     one DVE scalar_tensor_tensor; V carries a ones column (denominator).
  C. Per (qblk=512, group): 16 paired half-array score matmuls (two heads
     on partition halves 0/64) -> exp on ScalarE (scale=1/8 folded, no
     row max: |s/8| < ~6) -> ctx accumulated over t with V_aug stationary
     -> psum -> bf16 -> XBAR transpose back to [q,d] -> divide by the
     ones-column denominator on DVE -> f32 out tiles -> DMA out.
"""

import numpy as np

import concourse.bass as bass
import concourse.mybir as mybir
import concourse.tile as tile
from concourse import bacc, bass_utils

S = 2048
E = 1024
HL = 4          # heads per core
DH = 64         # head dim
NG = 2          # head groups per core (2 heads each -> 128 partitions)
EJ = E // 128   # 8 contraction tiles
ST = S // 128   # 16 sequence tiles
QB = 512        # projection / attention q block
F32 = mybir.dt.float32
BF16 = mybir.dt.bfloat16


def build_program(nc, reps=1, stages="all"):
    hid = nc.dram_tensor("hid", [S, E], F32, kind="ExternalInput")
    wq = nc.dram_tensor("wq", [HL * DH, E], F32, kind="ExternalInput")
    wk = nc.dram_tensor("wk", [HL * DH, E], F32, kind="ExternalInput")
    wv = nc.dram_tensor("wv", [HL * DH, E], F32, kind="ExternalInput")
    ws = nc.dram_tensor("ws", [64, E], F32, kind="ExternalInput")
    bq = nc.dram_tensor("bq", [HL * DH], F32, kind="ExternalInput")
    bk = nc.dram_tensor("bk", [HL * DH], F32, kind="ExternalInput")
    bv = nc.dram_tensor("bv", [HL * DH], F32, kind="ExternalInput")
    bs = nc.dram_tensor("bs", [64], F32, kind="ExternalInput")
    out = nc.dram_tensor("out", [S, HL * DH], F32, kind="ExternalOutput")

    AF = mybir.ActivationFunctionType
    ALU = mybir.AluOpType

    with tile.TileContext(nc) as tc:
        def emit(pfx):
            with (
                tc.tile_pool(name=pfx + "const", bufs=1) as cpool,
                tc.tile_pool(name=pfx + "qkv", bufs=1) as qkv,
                tc.tile_pool(name=pfx + "outp", bufs=4) as outp,
            ):
                bqT = cpool.tile([128, NG], F32, tag="bqT", name=pfx + "bqT")
                bkT = cpool.tile([128, NG], F32, tag="bkT", name=pfx + "bkT")
                bsT = cpool.tile([64, 1], F32, tag="bsT", name=pfx + "bsT")
                bv_rep = cpool.tile([128, HL * DH], F32, tag="bv_rep", name=pfx + "bv_rep")
                nc.sync.dma_start(bqT[:], bq.rearrange("(g p) -> p g", p=128))
                nc.sync.dma_start(bkT[:], bk.rearrange("(g p) -> p g", p=128))
                nc.sync.dma_start(bsT[:], bs.rearrange("(g p) -> p g", p=64))
                nc.sync.dma_start(
                    bv_rep[:], bv[None, :].to_broadcast((128, HL * DH))
                )

                # transposed operands, all bf16: [do, j, m] = X[m, 128j+do]
                hidT = qkv.tile([128, EJ, S], BF16, tag="hidT", name=pfx + "hidT")
                wqT = qkv.tile([128, EJ, HL * DH], BF16, tag="wqT", name=pfx + "wqT")
                wkT = qkv.tile([128, EJ, HL * DH], BF16, tag="wkT", name=pfx + "wkT")
                wvT = qkv.tile([128, EJ, HL * DH], BF16, tag="wvT", name=pfx + "wvT")
                wsT = qkv.tile([128, EJ, 64], BF16, tag="wsT", name=pfx + "wsT")

                QT = [qkv.tile([128, S], BF16, tag=f"QT{g}", name=f"{pfx}QT{g}") for g in range(NG)]
                KT = [qkv.tile([128, S], BF16, tag=f"KT{g}", name=f"{pfx}KT{g}") for g in range(NG)]
                VA = [qkv.tile([128, HL * 65], BF16, tag=f"VA{t}", name=f"{pfx}VA{t}") for t in range(ST)]
                sval = qkv.tile([64, S], BF16, tag="sval", name=pfx + "sval")
                srep = [qkv.tile([128, S], BF16, tag=f"srep{g}", name=f"{pfx}srep{g}") for g in range(NG)]

                # ones column of V_aug (denominator trick), set once per rep
                for t in range(ST):
                    va3 = VA[t].rearrange("p (h x) -> p h x", h=HL)
                    nc.gpsimd.memset(va3[:, :, 64], 1.0)

                with (
                    tc.tile_pool(name=pfx + "raw", bufs=4) as raw,
                    tc.tile_pool(name=pfx + "bfs", bufs=4) as bfs,
                    tc.tile_pool(name=pfx + "ppsum", bufs=6, space="PSUM") as ppsum,
                ):
                    # ---- stage A: weights -> bf16 -> XBAR transpose ----
                    for wdram, wT, rows in (
                        (wk, wkT, 256), (wv, wvT, 256), (wq, wqT, 256), (ws, wsT, 64),
                    ):
                        for h in range(rows // 128 if rows >= 128 else 1):
                            pr = min(128, rows)
                            wr = raw.tile([128, E], F32, tag="w_raw")
                            wb = bfs.tile([128, E], BF16, tag="w_bf")
                            nc.sync.dma_start(wr[0:pr, :], wdram[128 * h : 128 * h + pr, :])
                            nc.gpsimd.tensor_copy(wb[0:pr, :], wr[0:pr, :])
                            nc.sync.dma_start_transpose(
                                wT[:, :, pr * h : pr * (h + 1)], wb[0:pr, :]
                            )

                    # ---- stage A: hidden -> bf16 -> XBAR transpose ----
                    for i in range(ST):
                        hr = raw.tile([128, E], F32, tag="hid_raw")
                        hb = bfs.tile([128, E], BF16, tag="hid_bf")
                        nc.sync.dma_start(hr[:], hid[128 * i : 128 * (i + 1), :])
                        if i % 2 == 0:
                            nc.vector.tensor_copy(hb[:], hr[:])
                        else:
                            nc.gpsimd.tensor_copy(hb[:], hr[:])
                        nc.scalar.dma_start_transpose(
                            hidT[:, :, 128 * i : 128 * (i + 1)], hb[:]
                        )

                    # ---- stage B: K projection ----
                    for g in range(NG):
                        for qb in range(S // QB):
                            sl = slice(QB * qb, QB * (qb + 1))
                            psk = ppsum.tile([128, QB], F32, tag="psproj")
                            for j in range(EJ):
                                nc.tensor.matmul(
                                    psk[:],
                                    wkT[:, j, 128 * g : 128 * (g + 1)],
                                    hidT[:, j, sl],
                                    start=(j == 0),
                                    stop=(j == EJ - 1),
                                )
                            nc.vector.tensor_scalar_add(
                                KT[g][:, sl], psk[:], bkT[:, g : g + 1]
                            )

                    # ---- stage B: V projection ([t, d] layout + ones) ----
                    for t in range(ST):
                        psv = ppsum.tile([128, QB], F32, tag="psproj")
                        for j in range(EJ):
                            nc.tensor.matmul(
                                psv[:, 0 : HL * DH],
                                hidT[:, j, 128 * t : 128 * (t + 1)],
                                wvT[:, j, :],
                                start=(j == 0),
                                stop=(j == EJ - 1),
                            )
                        va3 = VA[t].rearrange("p (h x) -> p h x", h=HL)
                        nc.vector.tensor_tensor(
                            va3[:, :, 0:64],
                            psv[:, 0 : HL * DH].rearrange("p (h d) -> p h d", h=HL),
                            bv_rep.rearrange("p (h d) -> p h d", h=HL),
                            ALU.add,
                        )

                    # ---- stage B: group scales ----
                    for qb in range(S // QB):
                        sl = slice(QB * qb, QB * (qb + 1))
                        pss = ppsum.tile([128, QB], F32, tag="psproj")
                        for j in range(EJ):
                            nc.tensor.matmul(
                                pss[0:64, :],
                                wsT[:, j, :],
                                hidT[:, j, sl],
                                start=(j == 0),
                                stop=(j == EJ - 1),
                            )
                        nc.scalar.activation(
                            sval[:, sl], pss[0:64, :], AF.Sigmoid, bias=bsT[:, 0:1]
                        )
                        nc.vector.tensor_scalar(
                            sval[:, sl], sval[:, sl], 0.1, 0.95, ALU.mult, ALU.add
                        )
                        # expand [64, qb] -> [128, qb] per group: row p -> 32g + p//4
                        for g in range(NG):
                            nc.sync.dma_start(
                                srep[g][:, sl],
                                sval[32 * g : 32 * (g + 1), sl][:, None, :]
                                .to_broadcast((32, 4, QB)),
                            )

                    # ---- stage B: Q projection (bias + scale fused) ----
                    for g in range(NG):
                        for qb in range(S // QB):
                            sl = slice(QB * qb, QB * (qb + 1))
                            psq = ppsum.tile([128, QB], F32, tag="psproj")
                            for j in range(EJ):
                                nc.tensor.matmul(
                                    psq[:],
                                    wqT[:, j, 128 * g : 128 * (g + 1)],
                                    hidT[:, j, sl],
                                    start=(j == 0),
                                    stop=(j == EJ - 1),
                                )
                            nc.vector.scalar_tensor_tensor(
                                QT[g][:, sl],
                                psq[:],
                                bqT[:, g : g + 1],
                                srep[g][:, sl],
                                ALU.add,
                                ALU.mult,
                            )

                # ---- stage C: attention ----
                if stages == "proj":
                    return
                with (
                    tc.tile_pool(name=pfx + "expT", bufs=24) as expp,
                    tc.tile_pool(name=pfx + "ctxsb", bufs=4) as ctxp,
                    tc.tile_pool(name=pfx + "tpd", bufs=4) as tpd,
                    tc.tile_pool(name=pfx + "small", bufs=4) as small,
                    tc.tile_pool(name=pfx + "epsum", bufs=3, space="PSUM") as epsum,
                    tc.tile_pool(name=pfx + "psctx", bufs=2, space="PSUM") as psctx,
                ):
                    # cs staging rows 65:80 are transposed but never consumed;
                    # zero them once so the XBAR never reads uninitialized SBUF
                    cs_bufs = [
                        ctxp.tile([128, QB], BF16, tag=f"cs{i}", name=f"{pfx}cs{i}")
                        for i in range(4)
                    ]
                    for c in cs_bufs:
                        nc.gpsimd.memset(c[:], 0.0)

                    for qblk in range(S // QB):
                        qsl = slice(QB * qblk, QB * (qblk + 1))
                        outs = outp.tile(
                            [128, 4, HL * DH], F32, tag="out_sb",
                            name=f"{pfx}out_sb_{qblk}",
                        )
                        for g in range(NG):
                            ets = []
                            for t in range(ST):
                                pss = epsum.tile([128, 2 * QB], F32, tag="psbig")
                                for sub in range(2):
                                    hb = 64 * sub
                                    nc.tensor.matmul(
                                        pss[:, QB * sub : QB * (sub + 1)],
                                        KT[g][hb : hb + 64, 128 * t : 128 * (t + 1)],
                                        QT[g][hb : hb + 64, qsl],
                                        start=True,
                                        stop=True,
                                    )
                                et = expp.tile([128, 2 * QB], BF16, tag="expT")
                                nc.scalar.activation(et[:], pss[:], AF.Exp, scale=0.125)
                                ets.append(et)
                            for sub in range(2):
                                head = 2 * g + sub
                                psc = psctx.tile([65, QB], F32, tag="psc")
                                for t in range(ST):
                                    nc.tensor.matmul(
                                        psc[:],
                                        VA[t][:, 65 * head : 65 * (head + 1)],
                                        ets[t][:, QB * sub : QB * (sub + 1)],
                                        start=(t == 0),
                                        stop=(t == ST - 1),
                                    )
                                cs = cs_bufs[2 * g + sub]
                                nc.vector.tensor_copy(cs[0:65, :], psc[:])
                                pst = tpd.tile([128, 4, 128], BF16, tag="pst")
                                nc.sync.dma_start_transpose(pst[:], cs[:])
                                rec = small.tile([128, 4], F32, tag="rec")
                                nc.vector.reciprocal(
                                    rec[:],
                                    pst[:, :, 64:65].rearrange("p a b -> p (a b)"),
                                )
                                nc.vector.tensor_tensor(
                                    outs[:, :, DH * head : DH * (head + 1)],
                                    pst[:, :, 0:64],
                                    rec[:, :, None].to_broadcast((128, 4, 64)),
                                    ALU.mult,
                                )
                        nc.sync.dma_start(
                            out[qsl].rearrange("(qt p) c -> p qt c", p=128),
                            outs[:],
                        )

        for rep in range(reps):
            emit(f"R{rep}" if reps > 1 else "")
    return nc


_NC = None


def _get_compiled():
    global _NC
    if _NC is None:
        nc = bacc.Bacc(
            "TRN2",
            target_bir_lowering=False,
            debug=False,
            enable_asserts=False,
            num_devices=8,
        )
        build_program(nc)
        nc.compile()
        _NC = nc
    return _NC


def make_in_maps(hidden_states, Wq, bq, Wk, bk, Wv, bv, Ws, bs):
    c32 = lambda a: np.ascontiguousarray(a, dtype=np.float32)
    in_maps = []
    for c in range(8):
        b, hq = divmod(c, 4)
        r = slice(256 * hq, 256 * (hq + 1))
        rs = slice(64 * hq, 64 * (hq + 1))
        in_maps.append(
            {
                "hid": c32(hidden_states[b]),
                "wq": c32(Wq[r]), "bq": c32(bq[r]),
                "wk": c32(Wk[r]), "bk": c32(bk[r]),
                "wv": c32(Wv[r]), "bv": c32(bv[r]),
                "ws": c32(Ws[rs]), "bs": c32(bs[rs]),
            }
        )
    return in_maps


def assemble(results):
    out = np.empty((2, S, 1024), np.float32)
    for c in range(8):
        b, hq = divmod(c, 4)
        out[b, :, 256 * hq : 256 * (hq + 1)] = results[c]["out"]
    return out


def kernel(hidden_states, Wq, bq, Wk, bk, Wv, bv, Ws, bs):
    nc = _get_compiled()
    in_maps = make_in_maps(hidden_states, Wq, bq, Wk, bk, Wv, bv, Ws, bs)
    res = bass_utils.run_bass_kernel_spmd(nc, in_maps, core_ids=list(range(8)))
    return assemble(res.results)


# revision 16
# speedup vs baseline: 1.1942x; 1.1942x over previous
"""Trainium2 Bass kernel for CurvSelfAttention (B=2, S=2048, E=1024, H=16).

Sharding: 8 cores = 2 batches x 4 head-quads. Core c handles batch c//4 and
heads [4*(c%4), 4*(c%4)+4). Attention is head-independent, so there are no
collectives; each core gets its batch's hidden states and its heads' weight
row-slices, and returns a [S, 256] slice of the output.

v3 design, from on-silicon microbenchmarks:
  - ScalarE exp is the hard floor: 128 x [128,1024] exp tiles at ~1.15us
    each (1.2 GHz fixed). Everything else is scheduled around keeping
    ScalarE 100% busy on exp.
  - exp OUTPUT dtype f32r is the fast ACT path (1115ns vs 1337 f32 /
    1403 bf16 / 1521 fp16), so ets and the ctx matmuls are f32r
    (32/16-bit matmul mixing is not supported).
  - fp16 matmuls run at full rate (215ns/MM N=512) with near-exact
    products (fp16 mult -> f32 accumulate), so projections and scores
    use fp16 operands; transposes ride the DMA XBAR (2-byte only).
  - PE HAM re-throttles to 1.2 GHz after ~3.4us of idle. The attention
    loop therefore interleaves at t-tile granularity: scores(sec, t) +
    ctx(sec-1, t) keeps the PE active in every HAM window, and V/Q/s
    projection work is spread into the early sections.

Per-core program:
  A. hid/weights -> fp16 (DVE/GpSimd) -> [do, j, m] transposed layouts
     via XBAR (dma_start_transpose).
  B. K projection first (scores need full KT), then s/Q for qblk 0.
  C. 8 sections (qblk, g): 16 paired half-array score matmuls (two heads
     on partition halves 0/64) -> exp (scale=1/8, no row max) -> f32r ets;
     ctx for the previous section accumulates VA (f32r, ones column for
     the softmax denominator) over t interleaved with the scores; V and
     remaining s/Q projections fill sections 0-1; psum ctx -> fp16 ->
     XBAR transpose -> divide by denominator on DVE -> f32 out.
"""

import numpy as np

import concourse.bass as bass
import concourse.mybir as mybir
import concourse.tile as tile
from concourse import bacc, bass_utils

S = 2048
E = 1024
HL = 4          # heads per core
DH = 64         # head dim
NG = 2          # head groups per core (2 heads each -> 128 partitions)
EJ = E // 128   # 8 contraction tiles
ST = S // 128   # 16 sequence tiles
QB = 512        # projection / attention q block
NSEC = (S // QB) * NG
F32 = mybir.dt.float32
F32R = mybir.dt.float32r
FP16 = mybir.dt.float16


def build_program(nc, reps=1, stages="all"):
    hid = nc.dram_tensor("hid", [S, E], F32, kind="ExternalInput")
    wq = nc.dram_tensor("wq", [HL * DH, E], F32, kind="ExternalInput")
    wk = nc.dram_tensor("wk", [HL * DH, E], F32, kind="ExternalInput")
    wv = nc.dram_tensor("wv", [HL * DH, E], F32, kind="ExternalInput")
    ws = nc.dram_tensor("ws", [64, E], F32, kind="ExternalInput")
    bq = nc.dram_tensor("bq", [HL * DH], F32, kind="ExternalInput")
    bk = nc.dram_tensor("bk", [HL * DH], F32, kind="ExternalInput")
    bv = nc.dram_tensor("bv", [HL * DH], F32, kind="ExternalInput")
    bs = nc.dram_tensor("bs", [64], F32, kind="ExternalInput")
    out = nc.dram_tensor("out", [S, HL * DH], F32, kind="ExternalOutput")

    AF = mybir.ActivationFunctionType
    ALU = mybir.AluOpType

    with tile.TileContext(nc) as tc:
        def emit(pfx):
            with (
                tc.tile_pool(name=pfx + "const", bufs=1) as cpool,
                tc.tile_pool(name=pfx + "qkv", bufs=1) as qkv,
                tc.tile_pool(name=pfx + "raw", bufs=2) as raw,
                tc.tile_pool(name=pfx + "hbf", bufs=3) as hbf,
                tc.tile_pool(name=pfx + "outp", bufs=2) as outp,
                tc.tile_pool(name=pfx + "expT", bufs=17) as expp,
                tc.tile_pool(name=pfx + "ctxsb", bufs=1) as ctxp,
                tc.tile_pool(name=pfx + "tpd", bufs=4) as tpd,
                tc.tile_pool(name=pfx + "small", bufs=4) as small,
                tc.tile_pool(name=pfx + "ppsum", bufs=2, space="PSUM") as ppsum,
                tc.tile_pool(name=pfx + "epsum", bufs=2, space="PSUM") as epsum,
                tc.tile_pool(name=pfx + "psctx", bufs=2, space="PSUM") as psctx,
            ):
                bqT = cpool.tile([128, NG], F32, tag="bqT", name=pfx + "bqT")
                bkT = cpool.tile([128, NG], F32, tag="bkT", name=pfx + "bkT")
                bsT = cpool.tile([64, 1], F32, tag="bsT", name=pfx + "bsT")
                bv_rep = cpool.tile([128, HL * DH], F32, tag="bv_rep", name=pfx + "bv_rep")
                nc.sync.dma_start(bqT[:], bq.rearrange("(g p) -> p g", p=128))
                nc.sync.dma_start(bkT[:], bk.rearrange("(g p) -> p g", p=128))
                nc.sync.dma_start(bsT[:], bs.rearrange("(g p) -> p g", p=64))
                nc.sync.dma_start(
                    bv_rep[:], bv[None, :].to_broadcast((128, HL * DH))
                )

                # transposed fp16 operands: [do, j, m] = X[m, 128j+do]
                hidT = qkv.tile([128, EJ, S], FP16, tag="hidT", name=pfx + "hidT")
                wqT = qkv.tile([128, EJ, HL * DH], FP16, tag="wqT", name=pfx + "wqT")
                wkT = qkv.tile([128, EJ, HL * DH], FP16, tag="wkT", name=pfx + "wkT")
                wvT = qkv.tile([128, EJ, HL * DH], FP16, tag="wvT", name=pfx + "wvT")
                wsT = qkv.tile([128, EJ, 64], FP16, tag="wsT", name=pfx + "wsT")

                QT = [qkv.tile([128, S], FP16, tag=f"QT{g}", name=f"{pfx}QT{g}") for g in range(NG)]
                KT = [qkv.tile([128, S], FP16, tag=f"KT{g}", name=f"{pfx}KT{g}") for g in range(NG)]
                VA = [qkv.tile([128, HL * 65], F32R, tag=f"VA{t}", name=f"{pfx}VA{t}") for t in range(ST)]
                sval = qkv.tile([64, S], FP16, tag="sval", name=pfx + "sval")
                srep = [qkv.tile([128, S], FP16, tag=f"srep{g}", name=f"{pfx}srep{g}") for g in range(NG)]

                for t in range(ST):
                    va3 = VA[t].bitcast(F32).rearrange("p (h x) -> p h x", h=HL)
                    nc.gpsimd.memset(va3[:, :, 64], 1.0)

                # ---- weights -> fp16 -> XBAR transpose ----
                # wk first (K proj gates everything); wq/ws on gpsimd
                for wdram, wT, rows, eng in (
                    (wk, wkT, 256, nc.vector),
                    (wq, wqT, 256, nc.gpsimd),
                    (ws, wsT, 64, nc.gpsimd),
                    (wv, wvT, 256, nc.vector),
                ):
                    nh = max(1, rows // 128)
                    for h in range(nh):
                        pr = min(128, rows)
                        wr = raw.tile([128, E], F32, tag="w_raw")
                        wb = hbf.tile([128, E], FP16, tag="w_bf")
                        nc.sync.dma_start(wr[0:pr, :], wdram[128 * h : 128 * h + pr, :])
                        eng.tensor_copy(wb[0:pr, :], wr[0:pr, :])
                        nc.sync.dma_start_transpose(
                            wT[:, :, pr * h : pr * (h + 1)], wb[0:pr, :]
                        )

                # ---- hidden -> fp16 -> XBAR transpose ----
                for i in range(ST):
                    hr = raw.tile([128, E], F32, tag="hid_raw")
                    hb = hbf.tile([128, E], FP16, tag="hid_bf")
                    nc.sync.dma_start(hr[:], hid[128 * i : 128 * (i + 1), :])
                    nc.vector.tensor_copy(hb[:], hr[:])
                    nc.scalar.dma_start_transpose(
                        hidT[:, :, 128 * i : 128 * (i + 1)], hb[:]
                    )

                def k_proj(qb):
                    sl = slice(QB * qb, QB * (qb + 1))
                    for g in range(NG):
                        psk = ppsum.tile([128, QB], F32, tag="psproj")
                        for j in range(EJ):
                            nc.tensor.matmul(
                                psk[:],
                                wkT[:, j, 128 * g : 128 * (g + 1)],
                                hidT[:, j, sl],
                                start=(j == 0),
                                stop=(j == EJ - 1),
                            )
                        nc.vector.tensor_scalar_add(
                            KT[g][:, sl], psk[:], bkT[:, g : g + 1]
                        )

                def s_proj(qb):
                    sl = slice(QB * qb, QB * (qb + 1))
                    pss = ppsum.tile([128, QB], F32, tag="psproj")
                    for j in range(EJ):
                        nc.tensor.matmul(
                            pss[0:64, :],
                            wsT[:, j, :],
                            hidT[:, j, sl],
                            start=(j == 0),
                            stop=(j == EJ - 1),
                        )
                    nc.scalar.activation(
                        sval[:, sl], pss[0:64, :], AF.Sigmoid, bias=bsT[:, 0:1]
                    )
                    nc.vector.tensor_scalar(
                        sval[:, sl], sval[:, sl], 0.1, 0.95, ALU.mult, ALU.add
                    )
                    for g in range(NG):
                        nc.sync.dma_start(
                            srep[g][:, sl],
                            sval[32 * g : 32 * (g + 1), sl][:, None, :]
                            .to_broadcast((32, 4, QB)),
                        )

                def q_proj(qb, g):
                    sl = slice(QB * qb, QB * (qb + 1))
                    psq = ppsum.tile([128, QB], F32, tag="psproj")
                    for j in range(EJ):
                        nc.tensor.matmul(
                            psq[:],
                            wqT[:, j, 128 * g : 128 * (g + 1)],
                            hidT[:, j, sl],
                            start=(j == 0),
                            stop=(j == EJ - 1),
                        )
                    nc.vector.scalar_tensor_tensor(
                        QT[g][:, sl],
                        psq[:],
                        bqT[:, g : g + 1],
                        srep[g][:, sl],
                        ALU.add,
                        ALU.mult,
                    )

                def v_proj(t):
                    psv = ppsum.tile([128, QB], F32, tag="psproj")
                    for j in range(EJ):
                        nc.tensor.matmul(
                            psv[:, 0 : HL * DH],
                            hidT[:, j, 128 * t : 128 * (t + 1)],
                            wvT[:, j, :],
                            start=(j == 0),
                            stop=(j == EJ - 1),
                        )
                    va3 = VA[t].rearrange("p (h x) -> p h x", h=HL)
                    nc.vector.tensor_tensor(
                        va3[:, :, 0:64],
                        psv[:, 0 : HL * DH].rearrange("p (h d) -> p h d", h=HL),
                        bv_rep.rearrange("p (h d) -> p h d", h=HL),
                        ALU.add,
                    )

                # K projection fully (scores(t) needs KT column block t//4)
                for qb in range(S // QB):
                    k_proj(qb)
                s_proj(0)
                q_proj(0, 0)
                q_proj(0, 1)

                if stages == "proj":
                    for qb in range(1, S // QB):
                        s_proj(qb)
                        q_proj(qb, 0)
                        q_proj(qb, 1)
                    for t in range(ST):
                        v_proj(t)
                    return

                cs_bufs = [
                    ctxp.tile([128, QB], FP16, tag=f"cs{i}", name=f"{pfx}cs{i}")
                    for i in range(4)
                ]
                for c in cs_bufs:
                    nc.gpsimd.memset(c[:], 0.0)

                # section s = (qblk, g); scores/exp(sec) interleaved with
                # ctx(sec-1) at t granularity to keep the PE HAM-warm
                all_ets = {}
                psc_live = {}
                outs_live = {}

                def scores_step(sec, t):
                    qblk, g = divmod(sec, NG)
                    qsl = slice(QB * qblk, QB * (qblk + 1))
                    pss = epsum.tile([128, 2 * QB], F32, tag="psbig")
                    for sub in range(2):
                        hb = 64 * sub
                        nc.tensor.matmul(
                            pss[:, QB * sub : QB * (sub + 1)],
                            KT[g][hb : hb + 64, 128 * t : 128 * (t + 1)],
                            QT[g][hb : hb + 64, qsl],
                            start=True,
                            stop=True,
                        )
                    et = expp.tile([128, 2 * QB], F32R, tag="expT")
                    nc.scalar.activation(et[:], pss[:], AF.Exp, scale=0.125)
                    all_ets[(sec, t)] = et

                def ctx_step(sec, t):
                    for sub in range(2):
                        _, g = divmod(sec, NG)
                        head = 2 * g + sub
                        if t == 0:
                            psc_live[(sec, sub)] = psctx.tile(
                                [65, QB], F32, tag="psc", name=f"{pfx}psc{sec}_{sub}"
                            )
                        nc.tensor.matmul(
                            psc_live[(sec, sub)],
                            VA[t][:, 65 * head : 65 * (head + 1)],
                            all_ets[(sec, t)][:, QB * sub : QB * (sub + 1)],
                            start=(t == 0),
                            stop=(t == ST - 1),
                        )

                def ctx_finish(sec):
                    qblk, g = divmod(sec, NG)
                    if g == 0:
                        outs_live[qblk] = outp.tile(
                            [128, 4, HL * DH], F32, tag="out_sb",
                            name=f"{pfx}out_sb_{qblk}",
                        )
                    outs = outs_live[qblk]
                    for sub in range(2):
                        head = 2 * g + sub
                        psc = psc_live.pop((sec, sub))
                        cs = cs_bufs[2 * g + sub]
                        nc.vector.tensor_copy(cs[0:65, :], psc[:])
                        pst = tpd.tile([128, 4, 128], FP16, tag="pst")
                        nc.sync.dma_start_transpose(pst[:], cs[:])
                        rec = small.tile([128, 4], F32, tag="rec")
                        nc.vector.reciprocal(
                            rec[:],
                            pst[:, :, 64:65].rearrange("p a b -> p (a b)"),
                        )
                        nc.vector.tensor_tensor(
                            outs[:, :, DH * head : DH * (head + 1)],
                            pst[:, :, 0:64],
                            rec[:, :, None].to_broadcast((128, 4, 64)),
                            ALU.mult,
                        )
                    if g == NG - 1:
                        qsl = slice(QB * qblk, QB * (qblk + 1))
                        nc.sync.dma_start(
                            out[qsl].rearrange("(qt p) c -> p qt c", p=128),
                            outs[:],
                        )

                for sec in range(NSEC):
                    qblk = sec // NG
                    for t in range(ST):
                        scores_step(sec, t)
                        if sec == 0:
                            v_proj(t)
                        elif sec == 1 and t < 6 and t % 2 == 0:
                            # spread remaining s/Q projections into sec 1
                            qb = 1 + t // 2
                            s_proj(qb)
                            q_proj(qb, 0)
                            q_proj(qb, 1)
                        if sec >= 1:
                            ctx_step(sec - 1, t)
                            if t == ST - 1:
                                ctx_finish(sec - 1)
                            # free consumed exp tiles promptly
                            del all_ets[(sec - 1, t)]
                # last section has no successor to interleave with
                for t in range(ST):
                    ctx_step(NSEC - 1, t)
                ctx_finish(NSEC - 1)

        for rep in range(reps):
            emit(f"R{rep}" if reps > 1 else "")
    return nc


_NC = None


def _get_compiled():
    global _NC
    if _NC is None:
        nc = bacc.Bacc(
            "TRN2",
            target_bir_lowering=False,
            debug=False,
            enable_asserts=False,
            num_devices=8,
        )
        build_program(nc)
        nc.compile()
        _NC = nc
    return _NC


def make_in_maps(hidden_states, Wq, bq, Wk, bk, Wv, bv, Ws, bs):
    c32 = lambda a: np.ascontiguousarray(a, dtype=np.float32)
    in_maps = []
    for c in range(8):
        b, hq = divmod(c, 4)
        r = slice(256 * hq, 256 * (hq + 1))
        rs = slice(64 * hq, 64 * (hq + 1))
        in_maps.append(
            {
                "hid": c32(hidden_states[b]),
                "wq": c32(Wq[r]), "bq": c32(bq[r]),
                "wk": c32(Wk[r]), "bk": c32(bk[r]),
                "wv": c32(Wv[r]), "bv": c32(bv[r]),
                "ws": c32(Ws[rs]), "bs": c32(bs[rs]),
            }
        )
    return in_maps


def assemble(results):
    out = np.empty((2, S, 1024), np.float32)
    for c in range(8):
        b, hq = divmod(c, 4)
        out[b, :, 256 * hq : 256 * (hq + 1)] = results[c]["out"]
    return out


def kernel(hidden_states, Wq, bq, Wk, bk, Wv, bv, Ws, bs):
    nc = _get_compiled()
    in_maps = make_in_maps(hidden_states, Wq, bq, Wk, bk, Wv, bv, Ws, bs)
    res = bass_utils.run_bass_kernel_spmd(nc, in_maps, core_ids=list(range(8)))
    return assemble(res.results)
